# revision 108
# baseline (speedup 1.0000x reference)
"""Sliding-window (banded) attention for nn_AttLayer on 8 Trainium2 NeuronCores.

Reference computation (per window-block n of 512 positions, 64 blocks over L=32768):
  q/k/v = 1x1-conv projections of x1 (512ch -> 256ch)
  energy[l, m] = (q_block[:, l] . k_window[:, m]) / 16   over a 1024-wide window
  attn = softmax(energy + log(band_mask + 1e-6)) * band_mask
  out  = relu(v_window @ attn^T) -> 1x1-conv (256 -> 512) + bias, masked

Sharding: 64 blocks split contiguously across 8 cores (8 blocks each). Each core
gets a zero-padded halo slice of x1 and computes its 4096 output columns.

Kernel strategy (per core, SPMD — all per-core variation is in the data):
  - Projections on PE in fp8e4 DoubleRow perf mode (0.5 cycles/row, two
    128-channel contraction tiles per instruction -> 4x MAC throughput).
    x1 and the projection weights are split host-side into e4m3 hi/lo pairs
    (W scaled by 64 to center the fp8 range; the scaling folds into the exp
    scale and 1/64 into Wo). Three accumulation chains (hi*hi + lo*hi +
    hi*lo) recover ~bf16-level accuracy at 0.75x the f32r cycle cost.
  - One contiguous 4608-column k/v span per core (no halving): every block's
    window is a slice of it, so there are no halo stashes, no cross-half
    boundary, and the software pipeline never drains mid-kernel.
  - energy ALSO in fp8 DoubleRow: q/k are stored as device-split fp8 hi/lo
    pairs in pair layout [c_within, hl, cc, col], so energyT[m, l] =
    k_chunk^T q contracts all 256 channels per instruction; 3 chains at
    0.5 cycles/row = 0.75x the fp16 cost over the EXACT band intervals
    (2560 of 4096 window cols per block).  k's bias is dropped entirely —
    it only shifts each query's energy row by a constant, which softmax
    cancels exactly — so its hi/lo split is a plain scaled eviction
    (hi on ACT, lo on DVE); q keeps its bias via an fp16 scratch (DVE)
    split on Pool.  Projections for block b+1 run one block ahead so
    energy never waits on the cross-engine splits; within a block the
    chunks reading the freshly-evicted k-group go last (_EORDER).
  - exp on ScalarE from energy PSUM packed 5x[128,512] banks (one exp
    instruction per bank, pt at matching bank-major offsets).  Band
    masking: only each chunk's 128-wide diagonal strip can be
    out-of-band, so one [128,128] triangle-mask multiply per chunk on
    DVE (2-byte SBUF ops run at 2x); sequence-edge padding is baked into
    the first/last blocks' mask data so the program stays SPMD.
  - AV in [l, c] orientation: per 128-query chunk the valid keys span
    exactly 5 m-chunks; pt is the stationary operand and the moving vT
    tiles carry a 257th ones-column (zero-weight projection channel with
    bias 1.0), so column 256 of the AV PSUM accumulates the softmax
    denominator inside the same matmuls — the colsum matmul stream is
    gone.  Normalization+relu is then a per-partition tensor_scalar
    (max+mult by the [128,1] reciprocal) fused into the PSUM eviction.
  - The [c, l] layout the fp8 output projection needs is recreated by
    XBAR DMA transposes (one 3D-out descriptor per l-chunk) on the far
    off-critical-path DMA engines; the fp8 hi/lo split of relu runs on
    Pool, deferred one block so no compute queue ever head-of-line blocks
    on a fresh transpose.
  - Output projection in fp8 DoubleRow (host-split Wo hi/lo x relu hi/lo).
    Final bias/mask applied on host.
  - Three-deep software pipeline: block b's AV/relu/transpose (tail_a)
    are emitted after block b+1's energy, its relu fp8 split one block
    later, and its output projection (tail_b) one block later still.
  - The final block takes a fused low-latency fp16 path (colsum matmul +
    [c, l] AV with reordered accumulation, ACT/DVE-alternating evictions,
    drain-half outproj on the freed AV banks) because everything it
    exposes is pure tail latency.
"""

import numpy as np

NCORES = 8
L = 32768
CIN = 512
C = 256
BL = 512
HALF = 256
LC = L // NCORES              # 4096 positions per core
HALO = LC + 2 * HALF          # 4608 = k/v span per core
NB = 8                        # window blocks per core
WSCALE = 64.0                 # host-side fp8 scaling of Wq/Wk/Wv (and biases)
QK8S = 1.0 / 16.0             # q/k fp8 pair storage scale (values ~ +-20)
# softmax scale / W-scaling^2 / qk8 storage scale^2
EXP_SCALE = (1.0 / 16.0) / (WSCALE * WSCALE) / (QK8S * QK8S)

# Per m-chunk r (8 chunks of the 1024-wide window): EXACT valid l-interval
# (lo, width) within the block's 512 queries (fp16 has no min-width penalty).
INTERVALS = [
    (0, 128), (0, 256), (0, 384), (0, 512),
    (0, 512), (128, 384), (256, 256), (384, 128),
]
# accumulation order: r=3 covers the full [0,512) so it goes first (start=True)
AVORDER = [3, 4, 2, 5, 1, 6, 0, 7]

# Only the 128-wide diagonal strip of each m-chunk needs band masking (the
# rest of its l-interval is fully in-band): r<=3 mask their LAST 128 cols
# with the inclusive lower triangle, r>=4 their FIRST 128 with the strict
# upper triangle.  Sequence-edge padding needs full-tile masks for the
# first/last blocks' outer chunks (zeroed on the edge cores).
# cb16 layout: [ones(128) | woT0(512) | woT1(512) | tril(128) | triu(128)
#               | maskf r0,r1 (384) | maskl r6,r7 (384)]
_TRIL_OFF = 1152
_TRIU_OFF = 1280
_MF_OFF = [1408, 1536]                # widths 128, 256
_ML_OFF = [1792, 2048]                # widths 256, 128
CB16_W = 2176

# packed energy-PSUM banks: 2560 band columns fit 5 banks of 512, one exp
# instruction per bank; pt lives at matching bank-major offsets
_EBANK = {3: (0, 0), 4: (1, 0), 2: (2, 0), 0: (2, 384),
          5: (3, 0), 7: (3, 384), 1: (4, 0), 6: (4, 256)}
_PTOFF = {r: 512 * bk + off for r, (bk, off) in _EBANK.items()}
# energy emission order: the chunks reading the k-group evicted THIS
# iteration (r>=4) go last, giving its DVE lo-split time to land
_EORDER = [3, 2, 1, 0, 4, 5, 6, 7]
# exps emitted once the last chunk of a bank lands (emission in _EORDER);
# each bank's masks follow its exp
_EXP_AFTER = {3: [0], 0: [2], 4: [1], 6: [4], 7: [3]}
# bank allocation order = reverse completion order
_EALLOC = (3, 0, 2, 1, 4)


def _build_program():
    import concourse.mybir as mybir
    from concourse import bacc
    from concourse.tile import TileContext

    F32 = mybir.dt.float32
    F16 = mybir.dt.float16
    F8 = mybir.dt.float8e4
    BF16 = mybir.dt.bfloat16
    Alu = mybir.AluOpType
    Act = mybir.ActivationFunctionType
    PM = mybir.MatmulPerfMode.DoubleRow

    nc = bacc.Bacc()

    # x hi/lo fp8 halo slice in SBUF-tile order [c_within(128), hi/lo,
    # pair p, row j, pos] — hl leads the free dims so both full-piece and
    # hi/lo-sliced DMAs collapse to 3D APs; weights in DoubleRow pair
    # layout [c_in_within_chunk(128), hi/lo, pair p, row j, c_out] with
    # global input channel 128*(2p+j) + c_in.
    xhl_d = nc.dram_tensor("xhl", [128, 2, 2, 2, HALO], F8, kind="ExternalInput")
    w_d = {}
    for kind in ("q", "k"):
        w_d[kind] = nc.dram_tensor(
            f"w{kind}", [128, 2, 2, 2, C], F8, kind="ExternalInput")
    # v projection carries a 257th output channel (zero weights, bias 1.0):
    # the resulting ones-column of vT makes the AV matmul emit the softmax
    # denominator in the same PSUM tile, eliminating the colsum matmuls
    w_d["v"] = nc.dram_tensor("wv", [128, 2, 2, 2, C + 1], F8,
                              kind="ExternalInput")
    # f32 scalar blob: [bq0 bq1 bk0 bk1 (4 spare) | bvr(257)]
    cb32_d = nc.dram_tensor("cb32", [128, 266], F32, kind="ExternalInput")
    cb16_d = nc.dram_tensor("cb16", [128, CB16_W], F16, kind="ExternalInput")
    # fp8 hi/lo output-projection weights [c_in_within, hi/lo, cc, c_out]
    wo8_d = nc.dram_tensor("wo8", [128, 2, 2, CIN], F8, kind="ExternalInput")
    # c-major output: full channel g = 128*oc + c lives at [c, oc, :] — lets
    # one DMA carry several oc chunks (host re-interleaves)
    out_d = nc.dram_tensor("out", [128, 4, LC], F16, kind="ExternalOutput")

    with TileContext(nc) as tc:
        with (
            tc.tile_pool(name="consts", bufs=1) as consts,
            tc.tile_pool(name="xpool", bufs=1) as xpool,
            tc.tile_pool(name="qkv", bufs=1) as qkv,
            tc.tile_pool(name="ptp", bufs=2) as ptp,
            tc.tile_pool(name="sbo", bufs=4) as sbo,
            tc.tile_pool(name="sbod", bufs=6) as sbod,
            tc.tile_pool(name="pse", bufs=4, space="PSUM") as pse,
            tc.tile_pool(name="pav", bufs=2, space="PSUM") as pav,
            tc.tile_pool(name="pso", bufs=2, space="PSUM") as pso,
        ):
            # warm the PE clock gate (HAM) during the initial DMA wait:
            # dummy bf16 matmuls on memset data keep the array busy so the
            # first real projections run at the full 2.4 GHz; the memset
            # rides DVE (instantly idle) so the ramp starts as early as
            # possible — the 3us ramp threshold is absolute busy time
            warm_a = consts.tile([128, 128], BF16, name="warm_a")
            nc.vector.memset(warm_a, 1.0)

            # warm the ACT exp table while DMAs stream in
            warm_sb = consts.tile([1, 8], F32)
            nc.gpsimd.memset(warm_sb, 0.0)
            nc.scalar.activation(warm_sb, warm_sb, Act.Exp)
            warm_ps0 = pse.tile([128, 512], F32, tag="e", name="wps_first")
            for wj in range(3):
                nc.tensor.matmul(warm_ps0[:, 128 * wj:128 * (wj + 1)],
                                 warm_a, warm_a, start=True, stop=True)
            warm_b = consts.tile([128, 512], BF16, name="warm_b")
            nc.vector.memset(warm_b, 1.0)
            for wi in range(4):
                warm_ps = pse.tile([128, 512], F32, tag="e", name=f"wps{wi}")
                nc.tensor.matmul(warm_ps, warm_a, warm_b, start=True, stop=True)

            # critical-path-first DMA order: the first PE work is the
            # k-projection of columns [0:512), needing wk and x piece 0
            wT_sb = {}
            t = consts.tile([128, 2, 2, 2, C], F8, name="wk")
            nc.sync.dma_start(out=t, in_=w_d["k"].ap())
            wT_sb["k"] = t
            x_sb = xpool.tile([128, 2, 2, 2, HALO], F8, tag="x", name="x")

            def _x_cols(a, b):
                nc.sync.dma_start(
                    out=x_sb[:, :, :, :, a:b],
                    in_=xhl_d.ap()[:, :, :, :, a:b],
                )

            def _x_piece(ct):
                _x_cols(512 * ct, 512 * (ct + 1))

            def _x_half(ct, hl):
                nc.sync.dma_start(
                    out=x_sb[:, hl, :, :, 512 * ct:512 * (ct + 1)],
                    in_=xhl_d.ap()[:, hl, :, :, 512 * ct:512 * (ct + 1)],
                )

            _x_half(0, 0)
            _x_half(0, 1)
            _x_half(1, 0)
            _x_half(1, 1)
            t = consts.tile([128, 2, 2, 2, C], F8, name="wq")
            nc.sync.dma_start(out=t, in_=w_d["q"].ap())
            wT_sb["q"] = t
            t = consts.tile([128, 2, 2, 2, C + 1], F8, name="wv")
            nc.sync.dma_start(out=t, in_=w_d["v"].ap())
            wT_sb["v"] = t
            cb32_sb = consts.tile([128, 266], F32, name="cb32")
            nc.sync.dma_start(out=cb32_sb, in_=cb32_d.ap())
            bq_sb = [cb32_sb[:, 0:1], cb32_sb[:, 1:2]]
            bvrep_sb = cb32_sb[:, 8:265]
            _x_piece(2)
            _x_piece(3)
            _x_piece(4)

            cb16_sb = consts.tile([128, CB16_W], F16, name="cb16")
            nc.sync.dma_start(out=cb16_sb, in_=cb16_d.ap())
            ones_sb = cb16_sb[:, 0:128]
            woT_sb = [cb16_sb[:, 128:640], cb16_sb[:, 640:1152]]
            tril_sb = cb16_sb[:, _TRIL_OFF:_TRIL_OFF + 128]
            triu_sb = cb16_sb[:, _TRIU_OFF:_TRIU_OFF + 128]
            maskf_sb = [cb16_sb[:, _MF_OFF[i]:_MF_OFF[i] + INTERVALS[i][1]]
                        for i in range(2)]
            maskl_sb = [cb16_sb[:, _ML_OFF[i]:_ML_OFF[i] + INTERVALS[6 + i][1]]
                        for i in range(2)]
            wo8_sb = consts.tile([128, 2, 2, CIN], F8, name="wo8")
            nc.sync.dma_start(out=wo8_sb, in_=wo8_d.ap())

            # ---- projections (fp8 DoubleRow, 3 hi/lo chains) ----
            # q/k are stored as fp8 hi/lo pairs in DoubleRow pair layout
            # [c_within, hl, cc(j), col] so the energy contracts all 256
            # channels per instruction at 0.5 cycles/row.  k's bias is
            # dropped entirely: it only adds a per-query constant to the
            # energy rows, which softmax cancels exactly.
            k8_sb = [qkv.tile([128, 2, 2, 512], F8, tag=f"k8g{g}",
                              name=f"k8g{g}") for g in range(HALO // 512)]
            q8_sb = [qkv.tile([128, 2, 2, 512], F8, tag=f"q8g{g}",
                              name=f"q8g{g}") for g in range(LC // 512)]

            CHAINS = ((0, 0), (1, 0), (0, 1))  # (w hi/lo, x hi/lo)

            def _proj_psum(kind, cc, ps, x0):
                # accumulate W^T x into ps[128, 512] over K=512 via
                # 2 DoubleRow pair-steps x 3 chains x 2 col-halves
                csl = slice(128 * cc, 128 * (cc + 1))
                for half_i in range(2):
                    n0 = x0 + 256 * half_i
                    # x-lo chains last so projections start on the hi half
                    # of a freshly-split x DMA piece
                    order = [(0, 0, 0), (1, 0, 0), (0, 0, 1), (1, 0, 1),
                             (0, 1, 0), (0, 1, 1)]
                    for i, (wp, xp, p) in enumerate(order):
                        nc.tensor.matmul(
                            ps[:, 256 * half_i:256 * (half_i + 1)],
                            wT_sb[kind][:, wp, p, :, csl],
                            x_sb[:, xp, p, :, n0:n0 + 256],
                            start=(i == 0), stop=(i == len(order) - 1),
                            perf_mode=PM, skip_group_check=True,
                        )

            def k_group(mt):
                for cc in range(2):
                    ps = pse.tile([128, 512], F32, tag="e", name=f"psk{cc}{mt}")
                    _proj_psum("k", cc, ps, 512 * mt)
                    kg = k8_sb[mt]
                    # hi on ACT (scaled identity), lo on DVE: the two split
                    # halves land on different engines so neither queue
                    # serializes the projection stream
                    nc.scalar.activation(kg[:, 0, cc], ps,
                                         Act.Identity, scale=QK8S)
                    nc.vector.scalar_tensor_tensor(
                        kg[:, 1, cc], ps, QK8S, kg[:, 0, cc],
                        op0=Alu.mult, op1=Alu.subtract,
                    )

            def q_group(lt):
                for cc in range(2):
                    ps = pse.tile([128, 512], F32, tag="e", name=f"psq{cc}{lt}")
                    _proj_psum("q", cc, ps, HALF + 512 * lt)
                    # q keeps its bias (it does not cancel): fp16 scratch on
                    # DVE, then the fp8 hi/lo split rides the idle Pool.
                    # The first two groups gate block 0's energy during
                    # pipeline fill: they run in 256-col pieces with the two
                    # cc lanes' lo-splits spread across Pool and DVE
                    qg = q8_sb[lt]
                    npc = 2 if lt < 2 else 1
                    for pc in range(npc):
                        w = 512 // npc
                        sl = slice(w * pc, w * (pc + 1))
                        q16 = sbo.tile([128, w], F16, tag=f"q16{pc}",
                                       name=f"q16_{lt}{cc}{pc}")
                        nc.vector.tensor_scalar(
                            q16, ps[:, w * pc:w * (pc + 1)], QK8S, bq_sb[cc],
                            op0=Alu.mult, op1=Alu.add,
                        )
                        nc.scalar.activation(qg[:, 0, cc, sl], q16, Act.Copy)
                        lo_eng = nc.vector if (lt < 2 and cc == 1) else nc.gpsimd
                        lo_eng.tensor_tensor(
                            qg[:, 1, cc, sl], q16, qg[:, 0, cc, sl],
                            op=Alu.subtract,
                        )

            vT_sb = [None] * (HALO // 128)

            def vT_group(mts):
                for mt in mts:
                    ps = pso.tile([128, C + 1], F32, tag="o", name=f"psv{mt}")
                    first = True
                    for p in range(2):
                        for (wp, xp) in CHAINS:
                            nc.tensor.matmul(
                                ps,
                                x_sb[:, xp, p, :, 128 * mt:128 * (mt + 1)],
                                wT_sb["v"][:, wp, p],
                                start=first, stop=(p == 1 and (wp, xp) == CHAINS[-1]),
                                perf_mode=PM, skip_group_check=True,
                            )
                            first = False
                    t = qkv.tile([128, C + 2], F16, tag=f"v{mt}", name=f"vT{mt}")
                    # eviction with the (per-free-element) v bias folded in;
                    # col 256 becomes the ones-column (0-weight proj + 1.0 bias)
                    nc.vector.tensor_tensor(t[:, 0:C + 1], ps, bvrep_sb, op=Alu.add)
                    vT_sb[mt] = t

            # ---- attention blocks (software-pipelined: block b's AV/relu
            # are emitted after block b+1's energy, its relu fp8 split one
            # block later, its outproj one block later still) ----
            CHAINS_E = ((0, 0), (1, 0), (0, 1))  # (k hi/lo, q hi/lo)

            def emit_energy(b):
                woff = 512 * b   # window start in k/vT coords
                ptt = ptp.tile([128, 2560], F16, tag="pt", name=f"pt_{b}")
                banks = [None] * 5
                # allocation order = reverse completion order, so the next
                # block's projection PSUMs land on early-freed buffers
                for bk in _EALLOC:
                    banks[bk] = pse.tile([128, 512], F32, tag="e",
                                         name=f"pse{b}b{bk}")

                def _mask(r):
                    lo, w = INTERVALS[r]
                    off = _PTOFF[r]
                    eng = nc.vector
                    if b == 0 and r < 2:
                        eng.tensor_tensor(
                            ptt[:, off:off + w], ptt[:, off:off + w],
                            maskf_sb[r], op=Alu.mult)
                    elif b == NB - 1 and r >= 6:
                        eng.tensor_tensor(
                            ptt[:, off:off + w], ptt[:, off:off + w],
                            maskl_sb[r - 6], op=Alu.mult)
                    elif r <= 3:
                        # only the trailing 128-wide diagonal strip can be
                        # out-of-band; the rest of the interval is all-valid
                        sl = slice(off + w - 128, off + w)
                        eng.tensor_tensor(
                            ptt[:, sl], ptt[:, sl], tril_sb, op=Alu.mult)
                    else:
                        sl = slice(off, off + 128)
                        eng.tensor_tensor(
                            ptt[:, sl], ptt[:, sl], triu_sb, op=Alu.mult)

                for i, r in enumerate(_EORDER):
                    lo, w = INTERVALS[r]
                    bk, boff = _EBANK[r]
                    ps_e = banks[bk]
                    kg = k8_sb[b + r // 4]
                    koff = 128 * (r % 4)
                    # fp8 DoubleRow: both cc chunks contract per instruction
                    # at 0.5 cycles/row; <=256-wide pieces keep the moving
                    # free size within the 512 hardware limit
                    for p0 in range(0, w, 256):
                        p1 = min(p0 + 256, w)
                        for ci, (hk, hq) in enumerate(CHAINS_E):
                            nc.tensor.matmul(
                                ps_e[:, boff + p0:boff + p1],
                                kg[:, hk, :, koff:koff + 128],
                                q8_sb[b][:, hq, :, lo + p0:lo + p1],
                                start=(ci == 0), stop=(ci == 2),
                                perf_mode=PM, skip_group_check=True,
                            )
                    for ebk in _EXP_AFTER.get(r, ()):
                        nc.scalar.activation(
                            ptt[:, 512 * ebk:512 * (ebk + 1)], banks[ebk],
                            Act.Exp, scale=EXP_SCALE)
                        for rr, (bk2, _o) in _EBANK.items():
                            if bk2 == ebk:
                                _mask(rr)
                pt = {r: ptt[:, _PTOFF[r]:_PTOFF[r] + INTERVALS[r][1]]
                      for r in range(8)}
                return (b, pt)

            def emit_tail_a(ctx):
                # AV in [l, c] orientation: per 128-query chunk lc, the
                # valid keys span exactly 5 m-chunks (lc..lc+4).  pt is the
                # stationary operand; the moving vT tiles carry a 257th
                # ones-column, so column 256 of the PSUM accumulates the
                # softmax denominator in the same matmuls (no colsum).
                # Normalization becomes a per-partition tensor_scalar; the
                # [c, l] layout the outproj needs is recreated by XBAR DMA
                # transposes (DMA engines are far off the critical path).
                b, pt = ctx
                relu = sbo.tile([128, 4, 256], F16, tag="relu4",
                                name=f"relu{b}")
                reluT = sbod.tile([128, 2, 512], F16, tag="reluT",
                                 name=f"rT{b}")
                for lc in range(4):
                    ps_av = pav.tile([128, 512], F32, tag="av",
                                     name=f"psav{b}{lc}")
                    for s in range(5):
                        r = lc + s
                        lo, w = INTERVALS[r]
                        nc.tensor.matmul(
                            ps_av[:, 0:C + 1],
                            pt[r][:, 128 * lc - lo: 128 * lc - lo + 128],
                            vT_sb[4 * b + r][:, 0:C + 1],
                            start=(s == 0), stop=(s == 4),
                            skip_group_check=True,
                        )
                    rc = sbo.tile([128, 1], F32, tag=f"rcq{lc}",
                                  name=f"rc{b}{lc}")
                    nc.vector.reciprocal(rc, ps_av[:, C:C + 1])
                    # relu + normalization in one per-partition op
                    nc.vector.tensor_scalar(
                        relu[:, lc], ps_av[:, 0:C], 0.0, rc,
                        op0=Alu.max, op1=Alu.mult,
                    )
                    # one XBAR transpose per l-chunk: the 3D out AP's
                    # middle dim is logically the upper partition bits,
                    # landing both cc halves in reluT's [c, cc, l] layout
                    nc.sync.dma_start_transpose(
                        out=reluT[:, :, 128 * lc:128 * (lc + 1)],
                        in_=relu[:, lc, :],
                    )
                return (b, reluT)

            def emit_r8(tctx):
                # hi/lo fp8 split of reluT, deferred one block past the
                # transposes so no compute queue head-of-line blocks on a
                # fresh XBAR DMA; both splits on Pool (SBUF-only ops),
                # whose only consumer-side slack is a full block
                b, reluT = tctx
                use_fp8 = not (b == NB - 2)
                r8h = r8l = None
                if use_fp8:
                    r8h = sbod.tile([128, 2, 512], F8, tag="r8h",
                                   name=f"r8h{b}")
                    r8l = sbod.tile([128, 2, 512], F8, tag="r8l",
                                   name=f"r8l{b}")
                    nc.gpsimd.tensor_copy(r8h, reluT)
                    nc.gpsimd.tensor_tensor(r8l, reluT, r8h,
                                            op=Alu.subtract)
                return (b, reluT, r8h, r8l, use_fp8)

            def emit_tail_b(octx):
                b, reluT, r8h, r8l, use_fp8 = octx
                c0 = 512 * b
                o_sb = sbod.tile([128, 4, 512], F16, tag="osb", name=f"o{b}")
                for oc in range(4):
                    ps_o = pso.tile([128, 512], F32, tag="o",
                                    name=f"pso{b}{oc}")
                    if use_fp8:
                        for pc in range(2):
                            psl = slice(256 * pc, 256 * (pc + 1))
                            for ci, (wp, rp) in enumerate(
                                    ((0, r8h), (1, r8h), (0, r8l))):
                                nc.tensor.matmul(
                                    ps_o[:, psl],
                                    wo8_sb[:, wp, :, 128 * oc:128 * (oc + 1)],
                                    rp[:, :, psl],
                                    start=(ci == 0), stop=(ci == 2),
                                    perf_mode=PM, skip_group_check=True,
                                )
                        nc.scalar.activation(o_sb[:, oc], ps_o, Act.Copy,
                                             scale=1.0 / 1024.0)
                    else:
                        for cc in range(2):
                            nc.tensor.matmul(
                                ps_o, woT_sb[cc][:, 128 * oc:128 * (oc + 1)],
                                reluT[:, cc], start=(cc == 0), stop=(cc == 1),
                            )
                        nc.scalar.activation(o_sb[:, oc], ps_o, Act.Copy)
                # single DMA for all four oc chunks of this block
                nc.sync.dma_start(
                    out=out_d.ap()[:, :, c0:c0 + 512], in_=o_sb,
                )

            def emit_tail_last_colsum(ctx):
                # the final block's colsum runs BEFORE the drained outproj
                # queue: its PE work fills the wait on the previous
                # block's relu chain, and its reciprocal lands early
                b, pt = ctx
                ps_s = pav.tile([128, 512], F32, tag="av", name=f"pss{b}")
                for i, r in enumerate(AVORDER):
                    lo, w = INTERVALS[r]
                    nc.tensor.matmul(
                        ps_s[:, lo:lo + w], ones_sb, pt[r],
                        start=(i == 0), stop=(i == 7), skip_group_check=True,
                    )
                recip = sbo.tile([128, 512], F32, tag="recip", name=f"rc{b}")
                nc.vector.reciprocal(recip, ps_s)
                return recip

            def emit_tail_last(ctx, recip):
                # final block, fused fp16 path: AV order ending with the
                # l>=256 chunks so the left output half drains while AV
                # finishes — the pipeline is empty after this block and
                # every exposed serial step is pure tail latency
                b, pt = ctx
                avorder = [3, 4, 2, 5, 1, 0, 6, 7]
                ps_av = []
                for cc in range(2):
                    ps_av.append(pav.tile([128, 512], F32, tag="av",
                                          name=f"psavL{b}{cc}"))
                relu_sb = []
                for cc in range(2):
                    relu_sb.append(sbo.tile([128, 512], F16, tag=f"relu{cc}",
                                            name=f"relu{b}{cc}"))
                o_sb = sbod.tile([128, 4, 512], F16, tag="osb", name=f"o{b}")
                done = 0
                for (l0, l1), steps in (((0, 256), 6), ((256, 512), 8)):
                    for i in range(done, steps):
                        r = avorder[i]
                        lo, w = INTERVALS[r]
                        for cc in range(2):
                            nc.tensor.matmul(
                                ps_av[cc][:, lo:lo + w],
                                vT_sb[4 * b + r][:, 128 * cc:128 * (cc + 1)], pt[r],
                                start=(i == 0), stop=(i == steps - 1),
                                skip_group_check=True,
                            )
                    done = steps
                    lsl = slice(l0, l1)
                    c0 = 512 * b
                    for cc in range(2):
                        nc.vector.scalar_tensor_tensor(
                            relu_sb[cc][:, lsl], ps_av[cc][:, lsl], 0.0,
                            recip[:, lsl], op0=Alu.max, op1=Alu.mult,
                        )
                    for oc in range(4):
                        # the drain half borrows the freed colsum/AV banks
                        # for oc>=2 so all four accumulations overlap fully
                        pool_l = pav if (l0 > 0 and oc >= 2) else pso
                        ps_o = pool_l.tile([128, 512], F32,
                                           tag="av" if pool_l is pav else "o",
                                           name=f"pso{b}{oc}{l0}")
                        for cc in range(2):
                            nc.tensor.matmul(
                                ps_o[:, lsl],
                                woT_sb[cc][:, 128 * oc:128 * (oc + 1)],
                                relu_sb[cc][:, lsl],
                                start=(cc == 0), stop=(cc == 1),
                            )
                        # alternate ACT/DVE so the final evictions don't
                        # serialize on one engine
                        if oc % 2:
                            nc.vector.scalar_tensor_tensor(
                                o_sb[:, oc, lsl], ps_o[:, lsl], 0.0,
                                recip[:, lsl], op0=Alu.bypass, op1=Alu.bypass,
                            )
                        else:
                            nc.scalar.activation(o_sb[:, oc, lsl], ps_o[:, lsl],
                                                 Act.Copy)
                        # first half: oc-pair copies issue early under the
                        # remaining evictions; final half: one DMA so the
                        # terminal HWDGE/transfer chain is as short as
                        # possible
                        if l0 == 0 and oc % 2:
                            nc.sync.dma_start(
                                out=out_d.ap()[:, oc - 1:oc + 1, c0:c0 + 256],
                                in_=o_sb[:, oc - 1:oc + 1, 0:256],
                            )
                    if l0 > 0:
                        for ocp in (2, 0):
                            nc.sync.dma_start(
                                out=out_d.ap()[:, ocp:ocp + 2,
                                               c0 + 256:c0 + 512],
                                in_=o_sb[:, ocp:ocp + 2, 256:512],
                            )

            pending = []   # energy ctxs awaiting tail_a
            r8q = []       # tail_a outputs awaiting their deferred r8 split
            opending = []  # r8 outputs awaiting their deferred outproj
            for b in range(NB):
                # q/k for block b+1 are projected one block ahead: their
                # fp8 hi/lo splits cross two more engines than the old fp16
                # evictions, so energy(b) must never wait on them
                if b == 0:
                    k_group(0)
                    k_group(1)
                    q_group(0)
                    q_group(1)
                    vT_group(range(0, 8))
                else:
                    if b + 1 <= NB:
                        k_group(b + 1)
                    if b + 1 < NB:
                        q_group(b + 1)
                    vT_group(range(4 * b + 4, 4 * b + 8))
                # stream the remaining x pieces a few blocks ahead of the
                # k/v groups that consume them
                if 1 <= b <= 4:
                    _x_piece(b + 4)
                pending.append(emit_energy(b))
                if r8q:
                    opending.append(emit_r8(r8q.pop(0)))
                if len(pending) > 1:
                    r8q.append(emit_tail_a(pending.pop(0)))
                if len(opending) > 1:
                    emit_tail_b(opending.pop(0))
            # drain: the final block takes the fused low-latency path; the
            # queued outprojs for blocks 5/6 flush around its colsum
            ctx = pending.pop(0)
            recip = emit_tail_last_colsum(ctx)
            while r8q:
                opending.append(emit_r8(r8q.pop(0)))
            while opending:
                emit_tail_b(opending.pop(0))
            emit_tail_last(ctx, recip)
    nc.compile()
    return nc


_NC_CACHE = {}


def _get_nc():
    if "nc" not in _NC_CACHE:
        _NC_CACHE["nc"] = _build_program()
    return _NC_CACHE["nc"]


def _f8():
    try:
        import ml_dtypes
        return ml_dtypes.float8_e4m3
    except ImportError:  # pragma: no cover
        import jax.numpy as jnp
        return jnp.float8_e4m3


def _split8(a):
    f8 = _f8()
    hi = np.asarray(a, np.float32).astype(f8)
    lo = (np.asarray(a, np.float32) - hi.astype(np.float32)).astype(f8)
    return hi, lo


def make_in_maps(x1, mask, Wq, bq, Wk, bk, Wv, bv, Wo, bo):
    x1 = np.asarray(x1, dtype=np.float32).reshape(CIN, L)

    def _pairs(w):
        # (C_out, C_in=512) -> [128, 2(hl), 2(p), 2(j), C_out] DoubleRow pair
        # layout with global c_in = 128*(2p+j) + c_in_within
        ws = np.asarray(w, np.float32) * WSCALE
        co = ws.shape[0]
        hi, lo = _split8(ws.T)          # (512, C_out) each
        def lay(a):
            return a.reshape(2, 2, 128, co).transpose(2, 0, 1, 3)
        return np.ascontiguousarray(np.stack([lay(hi), lay(lo)], axis=1))

    wq8 = _pairs(Wq)
    wk8 = _pairs(Wk)
    # v gets a 257th output channel with zero weights; its 1.0 bias makes the
    # vT ones-column that yields the softmax denominator inside the AV matmul
    wv8 = _pairs(np.concatenate(
        [np.asarray(Wv, np.float32),
         np.zeros((1, CIN), np.float32)], axis=0))
    woT = (np.asarray(Wo, np.float32).T / WSCALE).astype(np.float16)
    # fp8 hi/lo of 16*Wo^T in [c_within, hl, cc, c_out] layout (the 16*64
    # product scaling is undone by the 2^-10 eviction scale)
    wo_hi, wo_lo = _split8(np.asarray(Wo, np.float32).T * 16.0)
    def _wo_lay(a):
        return a.reshape(2, 128, CIN).transpose(1, 0, 2)
    wo8 = np.ascontiguousarray(
        np.stack([_wo_lay(wo_hi), _wo_lay(wo_lo)], axis=1))

    cb32 = np.zeros((128, 266), np.float32)
    # bq at the q8 storage scale; bk is dropped entirely (it only shifts
    # each query's energy row by a constant, which softmax cancels)
    cb32[:, 0:2] = (np.asarray(bq, np.float32) * WSCALE * QK8S).reshape(2, 128).T
    cb32[:, 8:264] = np.broadcast_to(
        (np.asarray(bv, np.float32) * WSCALE).reshape(1, C), (128, C))
    cb32[:, 264] = 1.0   # vT ones-column bias (denominator channel)

    # band mask tiles: mask[m', j] = 1 iff the (key m = 128r+m', query
    # l = lo+j) pair is inside the sliding band; only the 128-wide diagonal
    # strips are ever out-of-band, needing just the two triangle patterns
    base_masks = []
    mrow = np.arange(128)[:, None]
    for r in range(8):
        lo, w = INTERVALS[r]
        ll = lo + np.arange(w)[None, :]
        mm = 128 * r + mrow
        base_masks.append(((ll <= mm) & (mm <= ll + BL - 1))
                          .astype(np.float16))

    cb16 = np.zeros((128, CB16_W), np.float16)
    cb16[:, 0:128] = 1.0
    cb16[:, 128:640] = woT[0:128]
    cb16[:, 640:1152] = woT[128:256]
    jj = np.arange(128)[None, :]
    cb16[:, _TRIL_OFF:_TRIL_OFF + 128] = (jj <= mrow).astype(np.float16)
    cb16[:, _TRIU_OFF:_TRIU_OFF + 128] = (jj > mrow).astype(np.float16)

    in_maps = []
    for c in range(NCORES):
        g0 = LC * c - HALF
        x1h = np.zeros((CIN, HALO), np.float32)
        s0, s1 = max(g0, 0), min(g0 + HALO, L)
        x1h[:, s0 - g0:s1 - g0] = x1[:, s0:s1]
        xh, xl = _split8(x1h)
        # [c_within, p, hl, j, pos] with global channel 128*(2p+j) + c_within
        xhl = np.stack([xh.reshape(2, 2, 128, HALO),
                        xl.reshape(2, 2, 128, HALO)], axis=0)
        xhl = np.ascontiguousarray(xhl.transpose(3, 0, 1, 2, 4))
        cb = cb16.copy()
        # sequence-edge padding baked into the first/last blocks' masks:
        # the first two key chunks of block 0 are pad on core 0, the last
        # two of block NB-1 are pad on core NCORES-1
        for i in range(2):
            lo, w = INTERVALS[i]
            cb[:, _MF_OFF[i]:_MF_OFF[i] + w] = (
                0.0 if c == 0 else base_masks[i])
            lo, w = INTERVALS[6 + i]
            cb[:, _ML_OFF[i]:_ML_OFF[i] + w] = (
                0.0 if c == NCORES - 1 else base_masks[6 + i])
        m = {
            "xhl": xhl,
            "wq": wq8, "wk": wk8, "wv": wv8, "wo8": wo8,
            "cb32": cb32, "cb16": cb,
        }
        in_maps.append(m)
    return in_maps


def postprocess(results, mask, bo):
    # per-core out is [128, 4, LC] c-major; channel g = 128*oc + c
    cols = np.concatenate(
        [np.asarray(results[c]["out"], np.float32).transpose(1, 0, 2)
         .reshape(CIN, LC) for c in range(NCORES)], axis=1)
    out = cols[None] + np.asarray(bo, np.float32)[None, :, None]
    return (out * np.asarray(mask, np.float32)).astype(np.float32)


def kernel(x1, x2, mask, Wq, bq, Wk, bk, Wv, bv, Wo, bo, **_unused):
    from concourse.bass_utils import run_bass_kernel_spmd

    nc = _get_nc()
    in_maps = make_in_maps(x1, mask, Wq, bq, Wk, bk, Wv, bv, Wo, bo)
    res = run_bass_kernel_spmd(nc, in_maps, core_ids=list(range(NCORES)))
    return postprocess(res.results, mask, bo)
